# revision 16
# baseline (speedup 1.0000x reference)
"""Trainium2 Bass kernel for nn_FANPhaseOffsetTransformerLayer.

Full inputs -> full output. Sharding: 8 cores; core c handles batch b=c//4
and sequence-row chunk qc=c%4 (512 rows) of that batch. Each core computes
q/k/v ONLY for its own 512 rows; k and v are then all-gathered across the
4 cores of each batch (replica groups [[0..3],[4..7]]) so every core holds
the full-batch K/V for attention over its own query rows. This removes the
4x-redundant K/V projections of the naive scheme (~95us of PE time/core).

Program is identical on all cores (SPMD): per-core differences enter only
through the input data (xqT/xres) and the collective's rank order.

Matmuls run in bf16 (fp32 PSUM accumulation); softmax scores in fp32,
softmax skips max-subtraction (scores bounded ~+-4), denominator comes from
a 65th ones-column appended to v. The Wo bias is folded into xres on the
host; LN uses bn_stats + Rsqrt; the FAN gate scale is fused into the
residual add.
"""

import math

import numpy as np
import ml_dtypes

B, S, D, H, E = 2, 2048, 1024, 16, 64
P_DIM, G_DIM = 256, 512
SC = 512  # rows per core
NCORES = 8
LN_EPS = 1e-5

_bf = ml_dtypes.bfloat16

_prog_cache = {}


def _build_program(gv: float, ln_triv=(False, False, False, False)):
    from contextlib import ExitStack

    import concourse.bass as bass
    import concourse.bacc as bacc
    import concourse.tile as tile
    import concourse.mybir as mybir

    f32 = mybir.dt.float32
    bf = mybir.dt.bfloat16
    AF = mybir.ActivationFunctionType
    ALU = mybir.AluOpType

    nc = bacc.Bacc(
        "TRN2",
        target_bir_lowering=False,
        debug=False,
        enable_asserts=False,
        num_devices=NCORES,
    )

    GROUPS = [[0, 1, 2, 3], [4, 5, 6, 7]]

    # ---------------- DRAM I/O ----------------
    d_xT = nc.dram_tensor("xT", [D, S], bf, kind="ExternalInput")
    d_xqT = nc.dram_tensor("xqT", [D, SC], bf, kind="ExternalInput")
    d_xres = nc.dram_tensor("xres", [SC, D], f32, kind="ExternalInput")
    d_wqT = nc.dram_tensor("wqT", [D, D], bf, kind="ExternalInput")
    d_wkT = nc.dram_tensor("wkT", [D, D], bf, kind="ExternalInput")
    d_wvT = nc.dram_tensor("wvT", [D, D], bf, kind="ExternalInput")
    d_woT = nc.dram_tensor("woT", [D, D], bf, kind="ExternalInput")
    d_wpT = nc.dram_tensor("wpT", [D, P_DIM], bf, kind="ExternalInput")
    d_wgT = nc.dram_tensor("wgT", [D, G_DIM], bf, kind="ExternalInput")
    d_bqc = nc.dram_tensor("bqc", [128, 8], f32, kind="ExternalInput")
    d_bkc = nc.dram_tensor("bkc", [128, 8], f32, kind="ExternalInput")
    d_bvf = nc.dram_tensor("bvf", [D], f32, kind="ExternalInput")
    d_bgf = nc.dram_tensor("bgf", [G_DIM], f32, kind="ExternalInput")
    d_ln1w = nc.dram_tensor("ln1w", [D], f32, kind="ExternalInput")
    d_ln1b = nc.dram_tensor("ln1b", [D], f32, kind="ExternalInput")
    d_ln2w = nc.dram_tensor("ln2w", [D], f32, kind="ExternalInput")
    d_ln2b = nc.dram_tensor("ln2b", [D], f32, kind="ExternalInput")
    d_offs = nc.dram_tensor("offs", [P_DIM], f32, kind="ExternalInput")
    d_offc = nc.dram_tensor("offc", [P_DIM], f32, kind="ExternalInput")
    d_sel = nc.dram_tensor("sel", [16, 16, 64], bf, kind="ExternalInput")
    d_ident = nc.dram_tensor("ident", [128, 128], bf, kind="ExternalInput")
    d_out = nc.dram_tensor("out", [SC, D], f32, kind="ExternalOutput")

    def bcast(handle, parts):
        ap_ = handle.ap()
        return bass.AP(
            tensor=ap_.tensor, offset=ap_.offset, ap=[[0, parts]] + list(ap_.ap)
        )

    def chunked(handle, nck, cols):
        """DRAM [D, cols] viewed as [128 parts][nck chunks][cols]."""
        ap_ = handle.ap()
        return bass.AP(
            tensor=ap_.tensor,
            offset=ap_.offset,
            ap=[[cols, 128], [128 * cols, nck], [1, cols]],
        )

    with tile.TileContext(nc, pool_alloc_mode="queue") as tc:
        with ExitStack() as ctx:
            misc1 = tc.alloc_tile_pool(name="misc1", bufs=1)
            kv = tc.alloc_tile_pool(name="kv", bufs=1, side="right")

            # ------- small constants needed in QKV phase -------
            bqc_sb = misc1.tile([128, 8], f32)
            nc.gpsimd.dma_start(out=bqc_sb, in_=d_bqc.ap())
            bkc_sb = misc1.tile([128, 8], f32)
            nc.gpsimd.dma_start(out=bkc_sb, in_=d_bkc.ap())
            bv_bc = misc1.tile([128, D], f32)
            nc.gpsimd.dma_start(out=bv_bc, in_=bcast(d_bvf, 128))
            eps_sb = misc1.tile([128, 1], f32)
            nc.vector.memset(eps_sb, LN_EPS)

            # ------- persistent attention-phase tiles -------
            qT_sb = kv.tile([128, 8, SC], bf)
            kT_sb = kv.tile([128, 8, S], bf)
            vaug = kv.tile([128, 16, 16, 65], bf)
            nc.vector.memset(vaug[:, :, :, 64:65], 1.0)



            # DRAM bounce buffers for the K all-gather
            dramp = tc.alloc_tile_pool(name="dramp", bufs=1, space="DRAM")
            cc_kin = dramp.tile([128, 8 * SC], bf)
            cc_kout = dramp.tile([4, 128, 8 * SC], bf)

            # raw attention output staging (lives through normalize)
            apo = tc.alloc_tile_pool(name="attnp", bufs=1, side="right")
            raw_sb = apo.tile([128, 16, 512], bf)
            den16 = apo.tile([16, 512], bf)
            rec16 = apo.tile([16, 512], bf)

            # ============ K (own chunk) + all-gather; q; V full-batch ============
            qkvw2 = tc.alloc_tile_pool(name="qkvw2", bufs=1, side="right")
            wv_sb = qkvw2.tile([128, 8, D], bf)
            xt_sb = qkvw2.tile([128, 8, S], bf)
            qkvw = tc.alloc_tile_pool(name="qkvw", bufs=1)
            wq_sb = qkvw.tile([128, 8, D], bf)
            wk_sb = qkvw.tile([128, 8, D], bf)
            xq_sb = qkvw.tile([128, 8, SC], bf)
            kTl = qkvw.tile([128, 8, SC], bf)
            # wk + xq first (k projection runs first), chunked per kc, on
            # separate queues so descriptor-gen parallelizes
            for kc in range(8):
                nc.scalar.dma_start(
                    out=xq_sb[:, kc, :], in_=d_xqT.ap()[kc * 128 : (kc + 1) * 128, :]
                )
                nc.sync.dma_start(
                    out=wk_sb[:, kc, :], in_=d_wkT.ap()[kc * 128 : (kc + 1) * 128, :]
                )
            nc.sync.dma_start(out=wq_sb, in_=chunked(d_wqT, 8, D))
            nc.gpsimd.dma_start(out=wv_sb, in_=chunked(d_wvT, 8, D))
            nc.gpsimd.dma_start(out=xt_sb, in_=chunked(d_xT, 8, S))

            with tc.tile_pool(name="ppq", bufs=1, space="PSUM") as ppq:
                # k^T for own chunk: kc-outer over all 8 PSUM banks
                psk = ppq.tile([128, 8, 512], f32, tag="big8", name="psk")
                for kc in range(8):
                    for m in range(8):
                        nc.tensor.matmul(
                            psk[:, m, :],
                            lhsT=wk_sb[:, kc, m * 128 : (m + 1) * 128],
                            rhs=xq_sb[:, kc, :],
                            start=(kc == 0),
                            stop=(kc == 7),
                        )
                for m in range(8):
                    nc.vector.tensor_scalar(
                        out=kTl[:, m, :],
                        in0=psk[:, m, :],
                        scalar1=bkc_sb[:, m : m + 1],
                        scalar2=None,
                        op0=ALU.add,
                    )
                nc.sync.dma_start(out=cc_kin, in_=kTl)

                # all-gather K across the 4 cores of this batch
                nc.gpsimd.collective_compute(
                    "AllGather",
                    mybir.AluOpType.bypass,
                    replica_groups=GROUPS,
                    ins=[cc_kin.opt()],
                    outs=[cc_kout.opt()],
                )

                # q^T for own chunk (overlaps the collective)
                psq = ppq.tile([128, 8, 512], f32, tag="big8", name="psq")
                for kc in range(8):
                    for m in range(8):
                        nc.tensor.matmul(
                            psq[:, m, :],
                            lhsT=wq_sb[:, kc, m * 128 : (m + 1) * 128],
                            rhs=xq_sb[:, kc, :],
                            start=(kc == 0),
                            stop=(kc == 7),
                        )
                for m in range(8):
                    nc.vector.tensor_scalar(
                        out=qT_sb[:, m, :],
                        in0=psq[:, m, :],
                        scalar1=bqc_sb[:, m : m + 1],
                        scalar2=None,
                        op0=ALU.add,
                    )

                # gather K back, quarter by quarter
                for r in range(4):
                    nc.sync.dma_start(
                        out=kT_sb[:, :, r * 512 : (r + 1) * 512],
                        in_=cc_kout[r, :, :],
                    )
            qkvw.release()

            # ================= attention (V emits interleaved) =================
            with tc.tile_pool(name="vps", bufs=1, space="PSUM") as vps, tc.tile_pool(
                name="ppa", bufs=1, space="PSUM"
            ) as ppa:

                def emit_v(tb):
                    for tm in range(4):
                        tcx = tb * 4 + tm
                        for h2 in range(2):
                            ps = vps.tile(
                                [128, 512], f32, tag="vp", bufs=2, name="vp"
                            )
                            for kc in range(8):
                                nc.tensor.matmul(
                                    ps,
                                    lhsT=xt_sb[
                                        :, kc, tb * 512 + tm * 128 : tb * 512 + (tm + 1) * 128
                                    ],
                                    rhs=wv_sb[:, kc, h2 * 512 : (h2 + 1) * 512],
                                    start=(kc == 0),
                                    stop=(kc == 7),
                                )
                            nc.vector.tensor_tensor(
                                out=vaug[:, tcx, h2 * 8 : (h2 + 1) * 8, 0:64],
                                in0=ps,
                                in1=bv_bc[:, h2 * 512 : (h2 + 1) * 512],
                                op=ALU.add,
                            )

                def attn_pair(p, q):
                    opsums = []
                    for j in range(2):
                        op = ppa.tile(
                            [65, 512], f32, tag=f"opsum{j}", bufs=1, name=f"opsum{j}"
                        )
                        opsums.append(op)
                    tbs = range(4 * q, 4 * q + 4)
                    for tb in tbs:
                        ps2 = ppa.tile(
                            [128, 2, 512], f32, tag="ps2", bufs=2, name="ps2"
                        )
                        for j in range(2):
                            off = j * 64
                            nc.tensor.matmul(
                                ps2[:, j, :],
                                lhsT=kT_sb[
                                    off : off + 64, p, tb * 128 : (tb + 1) * 128
                                ],
                                rhs=qT_sb[off : off + 64, p, :],
                                start=True,
                                stop=True,
                            )
                        probs = apo.tile(
                            [128, 2, 512], bf, tag="probs", bufs=3, name="probs"
                        )
                        nc.scalar.activation(
                            out=probs, in_=ps2, func=AF.Exp, scale=1.0 / math.sqrt(E)
                        )
                        for j in range(2):
                            nc.tensor.matmul(
                                opsums[j],
                                lhsT=vaug[:, tb, 2 * p + j, :],
                                rhs=probs[:, j, :],
                                start=(tb == 4 * q),
                                stop=(tb == 4 * q + 3),
                            )
                    for j in range(2):
                        if q == 0:
                            nc.vector.tensor_copy(
                                raw_sb[0:65, 2 * p + j, :], opsums[j]
                            )
                        else:
                            nc.vector.tensor_tensor(
                                out=raw_sb[0:65, 2 * p + j, :],
                                in0=opsums[j],
                                in1=raw_sb[0:65, 2 * p + j, :],
                                op=ALU.add,
                            )

                # v for quarter 0 up front; v for quarter q+1 interleaved
                # into attention on quarter q (PE filler under exp)
                emit_v(0)
                for q in range(4):
                    for p in range(8):
                        attn_pair(p, q)
                        if q < 3 and p == 1:
                            emit_v(q + 1)
            qkvw2.release()

            # ------- post-phase constants (loaded during attention) -------
            misc2 = tc.alloc_tile_pool(name="misc2", bufs=1)
            ln1w_bc = ln1b_bc = None
            if not (ln_triv[0] and ln_triv[1]):
                ln1w_bc = misc2.tile([128, D], f32)
                nc.gpsimd.dma_start(out=ln1w_bc, in_=bcast(d_ln1w, 128))
                ln1b_bc = misc2.tile([128, D], f32)
                nc.gpsimd.dma_start(out=ln1b_bc, in_=bcast(d_ln1b, 128))
            ln2w_bc = ln2b_bc = None
            if not (ln_triv[2] and ln_triv[3]):
                ln2w_bc = misc2.tile([128, D], f32)
                nc.gpsimd.dma_start(out=ln2w_bc, in_=bcast(d_ln2w, 128))
                ln2b_bc = misc2.tile([128, D], f32)
                nc.gpsimd.dma_start(out=ln2b_bc, in_=bcast(d_ln2b, 128))
            bg_bc = misc2.tile([128, G_DIM], f32)
            nc.gpsimd.dma_start(out=bg_bc, in_=bcast(d_bgf, 128))
            offs_bc = misc2.tile([128, P_DIM], f32)
            nc.gpsimd.dma_start(out=offs_bc, in_=bcast(d_offs, 128))
            offc_bc = misc2.tile([128, P_DIM], f32)
            nc.gpsimd.dma_start(out=offc_bc, in_=bcast(d_offc, 128))
            sel_sb = misc2.tile([16, 16, 64], bf)
            nc.gpsimd.dma_start(out=sel_sb, in_=d_sel.ap())
            ident_sb = misc2.tile([128, 128], bf)
            nc.gpsimd.dma_start(out=ident_sb, in_=d_ident.ap())
            xres_sb = misc2.tile([128, 4, D], f32)
            nc.sync.dma_start(
                out=xres_sb,
                in_=bass.AP(
                    tensor=d_xres.ap().tensor,
                    offset=0,
                    ap=[[D, 128], [128 * D, 4], [1, D]],
                ),
            )
            wo_sb = misc2.tile([128, 8, D], bf)
            nc.sync.dma_start(out=wo_sb, in_=chunked(d_woT, 8, D))
            attn_oT = misc2.tile([128, 8, SC], bf)
            odd_sb = misc2.tile([64, 8, 512], bf)

            # softmax denominators -> reciprocals
            nc.sync.dma_start(out=den16, in_=raw_sb[64:65, :, :])
            with nc.allow_low_precision(reason="softmax denominators: bf16 ample"):
                nc.vector.reciprocal(rec16, den16)

            # normalize; even heads direct, odd heads staged then shifted
            with tc.tile_pool(name="ppn", bufs=2, space="PSUM") as ppn:
                for h in range(16):
                    p_, j = h // 2, h % 2
                    div = ppn.tile([64, 512], f32, tag="div", name="div")
                    nc.tensor.matmul(
                        div,
                        lhsT=sel_sb[:, h, :],
                        rhs=rec16,
                        start=True,
                        stop=True,
                    )
                    if j == 0:
                        out_ap = attn_oT[0:64, p_, :]
                    else:
                        out_ap = odd_sb[0:64, p_, :]
                    nc.vector.tensor_tensor(
                        out=out_ap, in0=raw_sb[0:64, h, :], in1=div, op=ALU.mult
                    )
            nc.sync.dma_start(out=attn_oT[64:128, :, :], in_=odd_sb)
            apo.release()
            kv.release()

            # ================= Wo + LN1 + FAN + LN2 =================
            with tc.tile_pool(name="pw", bufs=1) as pw, tc.tile_pool(
                name="post", bufs=2
            ) as po, tc.tile_pool(name="ppp", bufs=2, space="PSUM") as ppp:
                wp_sb = pw.tile([128, 8, P_DIM], bf)
                nc.sync.dma_start(out=wp_sb, in_=chunked(d_wpT, 8, P_DIM))
                wg_sb = pw.tile([128, 8, G_DIM], bf)
                nc.sync.dma_start(out=wg_sb, in_=chunked(d_wgT, 8, G_DIM))
                z_sb = pw.tile([128, 4, D], f32, tag="zfan", name="z_sb")
                y_sb = pw.tile([128, 4, D], bf)
                yT_sb = pw.tile([128, 8, SC], bf)
                targ_sb = pw.tile([128, 4, 512], f32)
                g_sb = pw.tile([128, 4, 512], f32)

                def ln_stats(z_ap, tag):
                    """Return rsd [128,1], mean mv[:,0:1] for rows of z."""
                    stats = po.tile([128, 2, 6], f32, tag=f"lnst{tag}", name="lnst")
                    nc.vector.bn_stats(out=stats[:, 0, :], in_=z_ap[:, 0:512])
                    nc.vector.bn_stats(out=stats[:, 1, :], in_=z_ap[:, 512:1024])
                    mv = po.tile([128, 2], f32, tag=f"lnmv{tag}", name="lnmv")
                    nc.vector.bn_aggr(out=mv, in_=stats)
                    sd = po.tile([128, 2], f32, tag=f"lnsd{tag}", name="lnsd")
                    nc.scalar.activation(
                        out=sd[:, 0:1], in_=mv[:, 1:2], func=AF.Sqrt, bias=eps_sb
                    )
                    nc.vector.reciprocal(sd[:, 1:2], sd[:, 0:1])
                    return mv, sd[:, 1:2]

                def ln_apply(z_ap, w_bc, b_bc, out_ap, w_triv, b_triv, tag):
                    mv, rsd = ln_stats(z_ap, tag)
                    if w_triv and b_triv:
                        nc.vector.tensor_scalar(
                            out=out_ap,
                            in0=z_ap,
                            scalar1=mv[:, 0:1],
                            scalar2=rsd,
                            op0=ALU.subtract,
                            op1=ALU.mult,
                        )
                        return
                    tmp = po.tile([128, D], f32, tag=f"lntmp{tag}", name="lntmp")
                    nc.vector.tensor_scalar(
                        out=tmp,
                        in0=z_ap,
                        scalar1=mv[:, 0:1],
                        scalar2=rsd,
                        op0=ALU.subtract,
                        op1=ALU.mult,
                    )
                    if b_triv:
                        nc.vector.tensor_tensor(
                            out=out_ap, in0=tmp, in1=w_bc, op=ALU.mult
                        )
                        return
                    if not w_triv:
                        nc.vector.tensor_tensor(out=tmp, in0=tmp, in1=w_bc, op=ALU.mult)
                    nc.vector.tensor_tensor(out=out_ap, in0=tmp, in1=b_bc, op=ALU.add)

                # Wo projection + residual (+bias already folded into xres)
                for sc in range(4):
                    for h2 in range(2):
                        ps = ppp.tile([128, 512], f32, tag="wops", name="wops")
                        for kc in range(8):
                            nc.tensor.matmul(
                                ps,
                                lhsT=attn_oT[:, kc, sc * 128 : (sc + 1) * 128],
                                rhs=wo_sb[:, kc, h2 * 512 : (h2 + 1) * 512],
                                start=(kc == 0),
                                stop=(kc == 7),
                            )
                        nc.vector.tensor_tensor(
                            out=z_sb[:, sc, h2 * 512 : (h2 + 1) * 512],
                            in0=ps,
                            in1=xres_sb[:, sc, h2 * 512 : (h2 + 1) * 512],
                            op=ALU.add,
                        )
                for sc in range(4):
                    ln_apply(
                        z_sb[:, sc, :], ln1w_bc, ln1b_bc, y_sb[:, sc, :],
                        ln_triv[0], ln_triv[1], "a",
                    )

                # transpose y (bf16)
                for sc in range(4):
                    for dc in range(8):
                        tp = ppp.tile([128, 128], bf, tag="tp", name="tp")
                        nc.tensor.transpose(
                            tp, y_sb[:, sc, dc * 128 : (dc + 1) * 128], ident_sb
                        )
                        nc.vector.tensor_copy(
                            yT_sb[:, dc, sc * 128 : (sc + 1) * 128], tp
                        )

                # FAN sin branch: psp matmuls + offsets + range-reduce per sc
                RC = 12582912.0  # 1.5 * 2**23
                INV2PI = 1.0 / (2.0 * math.pi)
                for sc in range(4):
                    psp = ppp.tile([128, P_DIM], f32, tag="pps", name="pps")
                    for kc in range(8):
                        nc.tensor.matmul(
                            psp,
                            lhsT=yT_sb[:, kc, sc * 128 : (sc + 1) * 128],
                            rhs=wp_sb[:, kc, :],
                            start=(kc == 0),
                            stop=(kc == 7),
                        )
                    nc.vector.tensor_tensor(
                        out=targ_sb[:, sc, 0:256], in0=psp, in1=offs_bc, op=ALU.add
                    )
                    nc.vector.tensor_tensor(
                        out=targ_sb[:, sc, 256:512], in0=psp, in1=offc_bc, op=ALU.add
                    )
                    # range-reduce to [-pi, pi]: n = round(t/(2pi)); t -= 2pi*n
                    nred = po.tile([128, 512], f32, tag="nred", name="nred")
                    nc.vector.tensor_scalar(
                        out=nred,
                        in0=targ_sb[:, sc, :],
                        scalar1=INV2PI,
                        scalar2=RC,
                        op0=ALU.mult,
                        op1=ALU.add,
                    )
                    nc.vector.tensor_scalar(
                        out=nred,
                        in0=nred,
                        scalar1=RC,
                        scalar2=None,
                        op0=ALU.subtract,
                    )
                    nc.vector.scalar_tensor_tensor(
                        out=targ_sb[:, sc, :],
                        in0=nred,
                        scalar=-2.0 * math.pi,
                        in1=targ_sb[:, sc, :],
                        op0=ALU.mult,
                        op1=ALU.add,
                    )
                for sc in range(4):
                    nc.scalar.activation(
                        out=targ_sb[:, sc, :], in_=targ_sb[:, sc, :], func=AF.Sin
                    )

                # FAN gelu branch
                for sc in range(4):
                    psg = ppp.tile([128, G_DIM], f32, tag="ppg", name="ppg")
                    for kc in range(8):
                        nc.tensor.matmul(
                            psg,
                            lhsT=yT_sb[:, kc, sc * 128 : (sc + 1) * 128],
                            rhs=wg_sb[:, kc, :],
                            start=(kc == 0),
                            stop=(kc == 7),
                        )
                    nc.vector.tensor_tensor(
                        out=g_sb[:, sc, :], in0=psg, in1=bg_bc, op=ALU.add
                    )
                for sc in range(4):
                    nc.scalar.activation(
                        out=g_sb[:, sc, :], in_=g_sb[:, sc, :], func=AF.Gelu
                    )

                # z2 = y + gv*sin | y + (1-gv)*gelu, then LN2 + output
                for sc in range(4):
                    z2 = po.tile([128, D], f32, tag="z2", name="z2")
                    nc.vector.scalar_tensor_tensor(
                        out=z2[:, 0:512],
                        in0=targ_sb[:, sc, :],
                        scalar=float(gv),
                        in1=y_sb[:, sc, 0:512],
                        op0=ALU.mult,
                        op1=ALU.add,
                    )
                    nc.vector.scalar_tensor_tensor(
                        out=z2[:, 512:1024],
                        in0=g_sb[:, sc, :],
                        scalar=float(1.0 - gv),
                        in1=y_sb[:, sc, 512:1024],
                        op0=ALU.mult,
                        op1=ALU.add,
                    )
                    outt = po.tile([128, D], f32, tag="outt", name="outt")
                    ln_apply(
                        z2, ln2w_bc, ln2b_bc, outt, ln_triv[2], ln_triv[3], "b"
                    )
                    nc.sync.dma_start(
                        out=d_out.ap()[sc * 128 : (sc + 1) * 128, :], in_=outt
                    )

            misc2.release()
            dramp.release()
            misc1.release()

    nc.compile()
    return nc


def _host_inputs(inputs):
    """Build the per-core in_maps (list of 8 dicts) plus baked gate value."""
    f32 = np.float32
    x = np.asarray(inputs["x"], f32)
    Wq = np.asarray(inputs["Wq"], f32)
    Wk = np.asarray(inputs["Wk"], f32)
    Wv = np.asarray(inputs["Wv"], f32)
    Wo = np.asarray(inputs["Wo"], f32)
    Wp = np.asarray(inputs["Wp"], f32)
    Wg = np.asarray(inputs["Wg"], f32)
    bq = np.asarray(inputs["bq"], f32)
    bk = np.asarray(inputs["bk"], f32)
    bv = np.asarray(inputs["bv"], f32)
    bo = np.asarray(inputs["bo"], f32)
    bp = np.asarray(inputs["bp"], f32)
    bg = np.asarray(inputs["bg"], f32)
    offset = np.asarray(inputs["offset"], f32)
    gate = np.asarray(inputs["gate"], f32)
    ln1_w = np.asarray(inputs["ln1_w"], f32)
    ln1_b = np.asarray(inputs["ln1_b"], f32)
    ln2_w = np.asarray(inputs["ln2_w"], f32)
    ln2_b = np.asarray(inputs["ln2_b"], f32)

    gv = float(1.0 / (1.0 + np.exp(-gate[0])))

    sel = np.zeros((16, 16, 64), f32)
    for h in range(16):
        sel[h, h, :] = 1.0
    ident = np.eye(128, dtype=f32)

    shared = {
        "wqT": np.ascontiguousarray(Wq.T).astype(_bf),
        "wkT": np.ascontiguousarray(Wk.T).astype(_bf),
        "wvT": np.ascontiguousarray(Wv.T).astype(_bf),
        "woT": np.ascontiguousarray(Wo.T).astype(_bf),
        "wpT": np.ascontiguousarray(Wp.T).astype(_bf),
        "wgT": np.ascontiguousarray(Wg.T).astype(_bf),
        "bqc": np.ascontiguousarray(bq.reshape(8, 128).T),
        "bkc": np.ascontiguousarray(bk.reshape(8, 128).T),
        "bvf": bv,
        "bgf": bg,
        "ln1w": ln1_w,
        "ln1b": ln1_b,
        "ln2w": ln2_w,
        "ln2b": ln2_b,
        "offs": (offset + bp).astype(f32),
        "offc": (np.pi - offset + bp).astype(f32),
        "sel": sel.astype(_bf),
        "ident": ident.astype(_bf),
    }

    in_maps = []
    xT_by_b = [np.ascontiguousarray(x[b].T).astype(_bf) for b in range(B)]
    for c in range(NCORES):
        b, qc = c // 4, c % 4
        xT_b = xT_by_b[b]
        m = dict(shared)
        m["xT"] = xT_b
        m["xqT"] = np.ascontiguousarray(xT_b[:, qc * SC : (qc + 1) * SC])
        m["xres"] = np.ascontiguousarray(x[b, qc * SC : (qc + 1) * SC] + bo)
        in_maps.append(m)
    return in_maps, gv


def run(inputs, trace=False, tmpdir=None):
    """Run the kernel; returns (full_output, BassKernelResults)."""
    from concourse.bass_utils import run_bass_kernel_spmd

    in_maps, gv = _host_inputs(inputs)
    ln_triv = (
        bool(np.all(np.asarray(inputs["ln1_w"]) == 1.0)),
        bool(np.all(np.asarray(inputs["ln1_b"]) == 0.0)),
        bool(np.all(np.asarray(inputs["ln2_w"]) == 1.0)),
        bool(np.all(np.asarray(inputs["ln2_b"]) == 0.0)),
    )
    key = (round(gv, 9), ln_triv)
    if key not in _prog_cache:
        _prog_cache[key] = _build_program(gv, ln_triv)
    nc = _prog_cache[key]
    res = run_bass_kernel_spmd(
        nc, in_maps, core_ids=list(range(NCORES)), trace=trace, tmpdir=tmpdir
    )
    chunks = [res.results[c]["out"] for c in range(NCORES)]
    full = np.concatenate(chunks, axis=0).reshape(B, S, D).astype(np.float32)
    return full, res


def kernel(**inputs) -> np.ndarray:
    out, _ = run(inputs, trace=False)
    return out


# revision 17
# speedup vs baseline: 1.1117x; 1.1117x over previous
"""Trainium2 Bass kernel for nn_FANPhaseOffsetTransformerLayer.

Full inputs -> full output. Sharding: 8 cores; core c handles batch b=c//4
and sequence-row chunk qc=c%4 (512 rows) of that batch. Each core computes
k/v for its whole batch (redundant but fully hidden inside the exp-bound
attention window as PE filler), q only for its row chunk, attention for its
rows over all 16 heads, then Wo/LN1/FAN/LN2 for its rows.

Matmuls run in bf16 (fp32 PSUM accumulation) except the PV product, which
runs in fp8e4m3 with DoubleRow packing (two key blocks per matmul — 2x PE
throughput; probs are positive and bounded ~150, v is ~N(0, 0.4), both well
within e4m3 range). Softmax skips the max-subtraction; the denominator is
a 65th ones-column appended to v. Wo bias is folded into xres on the host;
the FAN gate scale is fused into the residual add; normalize uses bf16.
"""

import math

import numpy as np
import ml_dtypes

B, S, D, H, E = 2, 2048, 1024, 16, 64
P_DIM, G_DIM = 256, 512
SC = 512  # rows per core
NCORES = 8
LN_EPS = 1e-5

_bf = ml_dtypes.bfloat16

_prog_cache = {}


def _build_program(gv: float, ln_triv=(False, False, False, False)):
    from contextlib import ExitStack

    import concourse.bass as bass
    import concourse.bacc as bacc
    import concourse.tile as tile
    import concourse.mybir as mybir

    f32 = mybir.dt.float32
    bf = mybir.dt.bfloat16
    f8 = mybir.dt.float8e4
    AF = mybir.ActivationFunctionType
    ALU = mybir.AluOpType

    nc = bacc.Bacc(
        "TRN2",
        target_bir_lowering=False,
        debug=False,
        enable_asserts=False,
        num_devices=NCORES,
    )

    # ---------------- DRAM I/O ----------------
    d_xT = nc.dram_tensor("xT", [D, S], bf, kind="ExternalInput")
    d_xqT = nc.dram_tensor("xqT", [D, SC], bf, kind="ExternalInput")
    d_xres = nc.dram_tensor("xres", [SC, D], f32, kind="ExternalInput")
    d_wqT = nc.dram_tensor("wqT", [D, D], bf, kind="ExternalInput")
    d_wkT = nc.dram_tensor("wkT", [D, D], bf, kind="ExternalInput")
    d_wvT = nc.dram_tensor("wvT", [D, D], bf, kind="ExternalInput")
    d_woT = nc.dram_tensor("woT", [D, D], bf, kind="ExternalInput")
    d_wpT = nc.dram_tensor("wpT", [D, P_DIM], bf, kind="ExternalInput")
    d_wgT = nc.dram_tensor("wgT", [D, G_DIM], bf, kind="ExternalInput")
    d_bqc = nc.dram_tensor("bqc", [128, 8], f32, kind="ExternalInput")
    d_bkc = nc.dram_tensor("bkc", [128, 8], f32, kind="ExternalInput")
    d_bvf = nc.dram_tensor("bvf", [D], f32, kind="ExternalInput")
    d_bgf = nc.dram_tensor("bgf", [G_DIM], f32, kind="ExternalInput")
    d_ln1w = nc.dram_tensor("ln1w", [D], f32, kind="ExternalInput")
    d_ln1b = nc.dram_tensor("ln1b", [D], f32, kind="ExternalInput")
    d_ln2w = nc.dram_tensor("ln2w", [D], f32, kind="ExternalInput")
    d_ln2b = nc.dram_tensor("ln2b", [D], f32, kind="ExternalInput")
    d_offs = nc.dram_tensor("offs", [P_DIM], f32, kind="ExternalInput")
    d_offc = nc.dram_tensor("offc", [P_DIM], f32, kind="ExternalInput")
    d_sel = nc.dram_tensor("sel", [16, 16, 64], bf, kind="ExternalInput")
    d_ident = nc.dram_tensor("ident", [128, 128], bf, kind="ExternalInput")
    d_out = nc.dram_tensor("out", [SC, D], f32, kind="ExternalOutput")

    def bcast(handle, parts):
        ap_ = handle.ap()
        return bass.AP(
            tensor=ap_.tensor, offset=ap_.offset, ap=[[0, parts]] + list(ap_.ap)
        )

    def chunked(handle, nck, cols):
        ap_ = handle.ap()
        return bass.AP(
            tensor=ap_.tensor,
            offset=ap_.offset,
            ap=[[cols, 128], [128 * cols, nck], [1, cols]],
        )

    with tile.TileContext(nc, pool_alloc_mode="queue") as tc:
        with ExitStack() as ctx:
            misc1 = tc.alloc_tile_pool(name="misc1", bufs=1)

            bqc_sb = misc1.tile([128, 8], f32)
            nc.gpsimd.dma_start(out=bqc_sb, in_=d_bqc.ap())
            bkc_sb = misc1.tile([128, 8], f32)
            nc.gpsimd.dma_start(out=bkc_sb, in_=d_bkc.ap())
            bv_bc = misc1.tile([128, D], f32)
            nc.gpsimd.dma_start(out=bv_bc, in_=bcast(d_bvf, 128))
            eps_sb = misc1.tile([128, 1], f32)
            nc.vector.memset(eps_sb, LN_EPS)

            # persistent attention tiles (right side)
            kv = tc.alloc_tile_pool(name="kv", bufs=1, side="right")
            qT_sb = kv.tile([128, 8, SC], bf)
            kT_sb = kv.tile([128, 8, S], bf)
            # v in fp8, DoubleRow layout: [keys, tb-pair, parity, head, 80pad]
            vaug = kv.tile([128, 8, 2, 16, 80], f8)
            nc.vector.memset(vaug[:, :, :, :, 64:65], 1.0)

            # raw attention output staging (lives through normalize)
            apo = tc.alloc_tile_pool(name="attnp", bufs=1, side="right")
            raw_sb = apo.tile([128, 16, 512], bf)
            den16 = apo.tile([16, 512], bf)
            rec16 = apo.tile([16, 512], bf)

            # weights + x^T (live through attention; right side, freed after)
            qkvw2 = tc.alloc_tile_pool(name="qkvw2", bufs=1, side="right")
            wk_sb = qkvw2.tile([128, 8, D], bf)
            wv_sb = qkvw2.tile([128, 8, D], bf)
            xt_sb = qkvw2.tile([128, 8, S], bf)

            # q-projection operands (left, freed before the post phase)
            qkvw = tc.alloc_tile_pool(name="qkvw", bufs=1)
            wq_sb = qkvw.tile([128, 8, D], bf)
            xq_sb = qkvw.tile([128, 8, SC], bf)
            for kc in range(8):
                nc.scalar.dma_start(
                    out=xq_sb[:, kc, :], in_=d_xqT.ap()[kc * 128 : (kc + 1) * 128, :]
                )
                nc.sync.dma_start(
                    out=wq_sb[:, kc, :], in_=d_wqT.ap()[kc * 128 : (kc + 1) * 128, :]
                )
            nc.sync.dma_start(out=wk_sb, in_=chunked(d_wkT, 8, D))
            nc.gpsimd.dma_start(out=wv_sb, in_=chunked(d_wvT, 8, D))
            nc.gpsimd.dma_start(out=xt_sb, in_=chunked(d_xT, 8, S))

            # q^T for own chunk: kc-outer over all 8 PSUM banks
            with tc.tile_pool(name="ppq", bufs=1, space="PSUM") as ppq:
                psq = ppq.tile([128, 8, 512], f32, tag="big8", name="psq")
                for kc in range(8):
                    for m in range(8):
                        nc.tensor.matmul(
                            psq[:, m, :],
                            lhsT=wq_sb[:, kc, m * 128 : (m + 1) * 128],
                            rhs=xq_sb[:, kc, :],
                            start=(kc == 0),
                            stop=(kc == 7),
                        )
                for m in range(8):
                    nc.vector.tensor_scalar(
                        out=qT_sb[:, m, :],
                        in0=psq[:, m, :],
                        scalar1=bqc_sb[:, m : m + 1],
                        scalar2=None,
                        op0=ALU.add,
                    )
            qkvw.release()

            # ================= attention (k/v fills interleaved) =================
            with tc.tile_pool(name="fil", bufs=1, space="PSUM") as fil, tc.tile_pool(
                name="ppa", bufs=1, space="PSUM"
            ) as ppa:

                def emit_kt_block(tb, m):
                    ps = fil.tile([128, 512], f32, tag="fp", bufs=2, name="fp")
                    for kc in range(8):
                        nc.tensor.matmul(
                            ps,
                            lhsT=wk_sb[:, kc, m * 128 : (m + 1) * 128],
                            rhs=xt_sb[:, kc, tb * 512 : (tb + 1) * 512],
                            start=(kc == 0),
                            stop=(kc == 7),
                        )
                    nc.vector.tensor_scalar(
                        out=kT_sb[:, m, tb * 512 : (tb + 1) * 512],
                        in0=ps,
                        scalar1=bkc_sb[:, m : m + 1],
                        scalar2=None,
                        op0=ALU.add,
                    )

                def emit_v_block(tb, tm, h2):
                    tcx = tb * 4 + tm  # 128-row key block index (0..15)
                    ps = fil.tile([128, 512], f32, tag="fp", bufs=2, name="fp")
                    for kc in range(8):
                        nc.tensor.matmul(
                            ps,
                            lhsT=xt_sb[
                                :, kc, tb * 512 + tm * 128 : tb * 512 + (tm + 1) * 128
                            ],
                            rhs=wv_sb[:, kc, h2 * 512 : (h2 + 1) * 512],
                            start=(kc == 0),
                            stop=(kc == 7),
                        )
                    nc.vector.tensor_tensor(
                        out=vaug[:, tcx // 2, tcx % 2, h2 * 8 : (h2 + 1) * 8, 0:64],
                        in0=ps,
                        in1=bv_bc[:, h2 * 512 : (h2 + 1) * 512],
                        op=ALU.add,
                    )

                def quarter_fills(q):
                    """16 fill thunks that produce k^T and v for quarter q."""
                    fns = []
                    for m in range(8):
                        fns.append(lambda m=m: emit_kt_block(q, m))
                    for tm in range(4):
                        for h2 in range(2):
                            fns.append(lambda tm=tm, h2=h2: emit_v_block(q, tm, h2))
                    # interleave k and v emissions
                    out = []
                    for a, b2 in zip(fns[:8], fns[8:]):
                        out.append(a)
                        out.append(b2)
                    return out

                def attn_pair(p, q, fills):
                    opsums = []
                    for j in range(2):
                        op = ppa.tile(
                            [65, 512], f32, tag=f"opsum{j}", bufs=1, name=f"opsum{j}"
                        )
                        opsums.append(op)
                    for tp in range(2):
                        tbp = q * 2 + tp
                        probs = apo.tile(
                            [128, 2, 2, 512], f8, tag="probs", bufs=2, name="probs"
                        )
                        for parity in range(2):
                            tb = q * 4 + tp * 2 + parity
                            if fills:
                                fills.pop(0)()
                            ps2 = ppa.tile(
                                [128, 2, 512], f32, tag="ps2", bufs=2, name="ps2"
                            )
                            for j in range(2):
                                off = j * 64
                                nc.tensor.matmul(
                                    ps2[:, j, :],
                                    lhsT=kT_sb[
                                        off : off + 64, p, tb * 128 : (tb + 1) * 128
                                    ],
                                    rhs=qT_sb[off : off + 64, p, :],
                                    start=True,
                                    stop=True,
                                )
                            nc.scalar.activation(
                                out=probs[:, parity, :, :],
                                in_=ps2,
                                func=AF.Exp,
                                scale=1.0 / math.sqrt(E),
                            )
                        for j in range(2):
                            nc.tensor.matmul(
                                opsums[j],
                                lhsT=vaug[:, tbp, :, 2 * p + j, 0:65],
                                rhs=probs[:, :, j, :],
                                start=(tp == 0),
                                stop=(tp == 1),
                                perf_mode=mybir.MatmulPerfMode.DoubleRow,
                            )
                    for j in range(2):
                        if q == 0:
                            nc.vector.tensor_copy(
                                raw_sb[0:65, 2 * p + j, :], opsums[j]
                            )
                        else:
                            nc.vector.tensor_tensor(
                                out=raw_sb[0:65, 2 * p + j, :],
                                in0=opsums[j],
                                in1=raw_sb[0:65, 2 * p + j, :],
                                op=ALU.add,
                            )

                # quarter 0's k/v first, then attention with next-quarter fills
                for f in quarter_fills(0):
                    f()
                for q in range(4):
                    fills = quarter_fills(q + 1) if q < 3 else []
                    for p in range(8):
                        attn_pair(p, q, fills)
                    while fills:
                        fills.pop(0)()
            qkvw2.release()

            # ------- post-phase constants (loaded during attention) -------
            misc2 = tc.alloc_tile_pool(name="misc2", bufs=1)
            ln1w_bc = ln1b_bc = None
            if not (ln_triv[0] and ln_triv[1]):
                ln1w_bc = misc2.tile([128, D], f32)
                nc.gpsimd.dma_start(out=ln1w_bc, in_=bcast(d_ln1w, 128))
                ln1b_bc = misc2.tile([128, D], f32)
                nc.gpsimd.dma_start(out=ln1b_bc, in_=bcast(d_ln1b, 128))
            ln2w_bc = ln2b_bc = None
            if not (ln_triv[2] and ln_triv[3]):
                ln2w_bc = misc2.tile([128, D], f32)
                nc.gpsimd.dma_start(out=ln2w_bc, in_=bcast(d_ln2w, 128))
                ln2b_bc = misc2.tile([128, D], f32)
                nc.gpsimd.dma_start(out=ln2b_bc, in_=bcast(d_ln2b, 128))
            bg_bc = misc2.tile([128, G_DIM], f32)
            nc.gpsimd.dma_start(out=bg_bc, in_=bcast(d_bgf, 128))
            offs_bc = misc2.tile([128, P_DIM], f32)
            nc.gpsimd.dma_start(out=offs_bc, in_=bcast(d_offs, 128))
            offc_bc = misc2.tile([128, P_DIM], f32)
            nc.gpsimd.dma_start(out=offc_bc, in_=bcast(d_offc, 128))
            sel_sb = misc2.tile([16, 16, 64], bf)
            nc.gpsimd.dma_start(out=sel_sb, in_=d_sel.ap())
            ident_sb = misc2.tile([128, 128], bf)
            nc.gpsimd.dma_start(out=ident_sb, in_=d_ident.ap())
            xres_sb = misc2.tile([128, 4, D], f32)
            nc.sync.dma_start(
                out=xres_sb,
                in_=bass.AP(
                    tensor=d_xres.ap().tensor,
                    offset=0,
                    ap=[[D, 128], [128 * D, 4], [1, D]],
                ),
            )
            wo_sb = misc2.tile([128, 8, D], bf)
            nc.sync.dma_start(out=wo_sb, in_=chunked(d_woT, 8, D))
            attn_oT = misc2.tile([128, 8, SC], bf)
            odd_sb = misc2.tile([64, 8, 512], bf)

            # softmax denominators -> reciprocals
            nc.sync.dma_start(out=den16, in_=raw_sb[64:65, :, :])
            with nc.allow_low_precision(reason="softmax denominators: bf16 ample"):
                nc.vector.reciprocal(rec16, den16)

            # normalize; even heads direct, odd heads staged then shifted
            with tc.tile_pool(name="ppn", bufs=2, space="PSUM") as ppn:
                for h in range(16):
                    p_, j = h // 2, h % 2
                    div = ppn.tile([64, 512], f32, tag="div", name="div")
                    nc.tensor.matmul(
                        div,
                        lhsT=sel_sb[:, h, :],
                        rhs=rec16,
                        start=True,
                        stop=True,
                    )
                    if j == 0:
                        out_ap = attn_oT[0:64, p_, :]
                    else:
                        out_ap = odd_sb[0:64, p_, :]
                    nc.vector.tensor_tensor(
                        out=out_ap, in0=raw_sb[0:64, h, :], in1=div, op=ALU.mult
                    )
            nc.sync.dma_start(out=attn_oT[64:128, :, :], in_=odd_sb)
            apo.release()
            kv.release()

            # ================= Wo + LN1 + FAN + LN2 =================
            with tc.tile_pool(name="pw", bufs=1) as pw, tc.tile_pool(
                name="post", bufs=2
            ) as po, tc.tile_pool(name="ppp", bufs=2, space="PSUM") as ppp:
                wp_sb = pw.tile([128, 8, P_DIM], bf)
                nc.sync.dma_start(out=wp_sb, in_=chunked(d_wpT, 8, P_DIM))
                wg_sb = pw.tile([128, 8, G_DIM], bf)
                nc.sync.dma_start(out=wg_sb, in_=chunked(d_wgT, 8, G_DIM))
                z_sb = pw.tile([128, 4, D], f32, tag="zfan", name="z_sb")
                y_sb = pw.tile([128, 4, D], bf)
                yT_sb = pw.tile([128, 8, SC], bf)
                targ_sb = pw.tile([128, 4, 512], f32)
                g_sb = pw.tile([128, 4, 512], f32)

                def ln_stats(z_ap, tag):
                    stats = po.tile([128, 2, 6], f32, tag=f"lnst{tag}", name="lnst")
                    nc.vector.bn_stats(out=stats[:, 0, :], in_=z_ap[:, 0:512])
                    nc.vector.bn_stats(out=stats[:, 1, :], in_=z_ap[:, 512:1024])
                    mv = po.tile([128, 2], f32, tag=f"lnmv{tag}", name="lnmv")
                    nc.vector.bn_aggr(out=mv, in_=stats)
                    sd = po.tile([128, 2], f32, tag=f"lnsd{tag}", name="lnsd")
                    nc.scalar.activation(
                        out=sd[:, 0:1], in_=mv[:, 1:2], func=AF.Sqrt, bias=eps_sb
                    )
                    nc.vector.reciprocal(sd[:, 1:2], sd[:, 0:1])
                    return mv, sd[:, 1:2]

                def ln_apply(z_ap, w_bc, b_bc, out_ap, w_triv, b_triv, tag):
                    mv, rsd = ln_stats(z_ap, tag)
                    if w_triv and b_triv:
                        nc.vector.tensor_scalar(
                            out=out_ap,
                            in0=z_ap,
                            scalar1=mv[:, 0:1],
                            scalar2=rsd,
                            op0=ALU.subtract,
                            op1=ALU.mult,
                        )
                        return
                    tmp = po.tile([128, D], f32, tag=f"lntmp{tag}", name="lntmp")
                    nc.vector.tensor_scalar(
                        out=tmp,
                        in0=z_ap,
                        scalar1=mv[:, 0:1],
                        scalar2=rsd,
                        op0=ALU.subtract,
                        op1=ALU.mult,
                    )
                    if b_triv:
                        nc.vector.tensor_tensor(
                            out=out_ap, in0=tmp, in1=w_bc, op=ALU.mult
                        )
                        return
                    if not w_triv:
                        nc.vector.tensor_tensor(out=tmp, in0=tmp, in1=w_bc, op=ALU.mult)
                    nc.vector.tensor_tensor(out=out_ap, in0=tmp, in1=b_bc, op=ALU.add)

                # Wo projection + residual (+bias already folded into xres)
                for sc in range(4):
                    for h2 in range(2):
                        ps = ppp.tile([128, 512], f32, tag="wops", name="wops")
                        for kc in range(8):
                            nc.tensor.matmul(
                                ps,
                                lhsT=attn_oT[:, kc, sc * 128 : (sc + 1) * 128],
                                rhs=wo_sb[:, kc, h2 * 512 : (h2 + 1) * 512],
                                start=(kc == 0),
                                stop=(kc == 7),
                            )
                        nc.vector.tensor_tensor(
                            out=z_sb[:, sc, h2 * 512 : (h2 + 1) * 512],
                            in0=ps,
                            in1=xres_sb[:, sc, h2 * 512 : (h2 + 1) * 512],
                            op=ALU.add,
                        )
                for sc in range(4):
                    ln_apply(
                        z_sb[:, sc, :], ln1w_bc, ln1b_bc, y_sb[:, sc, :],
                        ln_triv[0], ln_triv[1], "a",
                    )

                # transpose y (bf16)
                for sc in range(4):
                    for dc in range(8):
                        tp = ppp.tile([128, 128], bf, tag="tp", name="tp")
                        nc.tensor.transpose(
                            tp, y_sb[:, sc, dc * 128 : (dc + 1) * 128], ident_sb
                        )
                        nc.vector.tensor_copy(
                            yT_sb[:, dc, sc * 128 : (sc + 1) * 128], tp
                        )

                # FAN sin branch
                RC = 12582912.0  # 1.5 * 2**23
                INV2PI = 1.0 / (2.0 * math.pi)
                for sc in range(4):
                    psp = ppp.tile([128, P_DIM], f32, tag="pps", name="pps")
                    for kc in range(8):
                        nc.tensor.matmul(
                            psp,
                            lhsT=yT_sb[:, kc, sc * 128 : (sc + 1) * 128],
                            rhs=wp_sb[:, kc, :],
                            start=(kc == 0),
                            stop=(kc == 7),
                        )
                    nc.vector.tensor_tensor(
                        out=targ_sb[:, sc, 0:256], in0=psp, in1=offs_bc, op=ALU.add
                    )
                    nc.vector.tensor_tensor(
                        out=targ_sb[:, sc, 256:512], in0=psp, in1=offc_bc, op=ALU.add
                    )
                    nred = po.tile([128, 512], f32, tag="nred", name="nred")
                    nc.vector.tensor_scalar(
                        out=nred,
                        in0=targ_sb[:, sc, :],
                        scalar1=INV2PI,
                        scalar2=RC,
                        op0=ALU.mult,
                        op1=ALU.add,
                    )
                    nc.vector.tensor_scalar(
                        out=nred, in0=nred, scalar1=RC, scalar2=None,
                        op0=ALU.subtract,
                    )
                    nc.vector.scalar_tensor_tensor(
                        out=targ_sb[:, sc, :],
                        in0=nred,
                        scalar=-2.0 * math.pi,
                        in1=targ_sb[:, sc, :],
                        op0=ALU.mult,
                        op1=ALU.add,
                    )
                for sc in range(4):
                    nc.scalar.activation(
                        out=targ_sb[:, sc, :], in_=targ_sb[:, sc, :], func=AF.Sin
                    )

                # FAN gelu branch
                for sc in range(4):
                    psg = ppp.tile([128, G_DIM], f32, tag="ppg", name="ppg")
                    for kc in range(8):
                        nc.tensor.matmul(
                            psg,
                            lhsT=yT_sb[:, kc, sc * 128 : (sc + 1) * 128],
                            rhs=wg_sb[:, kc, :],
                            start=(kc == 0),
                            stop=(kc == 7),
                        )
                    nc.vector.tensor_tensor(
                        out=g_sb[:, sc, :], in0=psg, in1=bg_bc, op=ALU.add
                    )
                for sc in range(4):
                    nc.scalar.activation(
                        out=g_sb[:, sc, :], in_=g_sb[:, sc, :], func=AF.Gelu
                    )

                # z2 = y + gv*sin | y + (1-gv)*gelu, then LN2 + output
                for sc in range(4):
                    z2 = po.tile([128, D], f32, tag="z2", name="z2")
                    nc.vector.scalar_tensor_tensor(
                        out=z2[:, 0:512],
                        in0=targ_sb[:, sc, :],
                        scalar=float(gv),
                        in1=y_sb[:, sc, 0:512],
                        op0=ALU.mult,
                        op1=ALU.add,
                    )
                    nc.vector.scalar_tensor_tensor(
                        out=z2[:, 512:1024],
                        in0=g_sb[:, sc, :],
                        scalar=float(1.0 - gv),
                        in1=y_sb[:, sc, 512:1024],
                        op0=ALU.mult,
                        op1=ALU.add,
                    )
                    outt = po.tile([128, D], f32, tag="outt", name="outt")
                    ln_apply(
                        z2, ln2w_bc, ln2b_bc, outt, ln_triv[2], ln_triv[3], sc % 2
                    )
                    nc.sync.dma_start(
                        out=d_out.ap()[sc * 128 : (sc + 1) * 128, :], in_=outt
                    )

            misc2.release()
            misc1.release()

    nc.compile()
    return nc


def _host_inputs(inputs):
    """Build the per-core in_maps (list of 8 dicts) plus baked gate value."""
    f32 = np.float32
    x = np.asarray(inputs["x"], f32)
    Wq = np.asarray(inputs["Wq"], f32)
    Wk = np.asarray(inputs["Wk"], f32)
    Wv = np.asarray(inputs["Wv"], f32)
    Wo = np.asarray(inputs["Wo"], f32)
    Wp = np.asarray(inputs["Wp"], f32)
    Wg = np.asarray(inputs["Wg"], f32)
    bq = np.asarray(inputs["bq"], f32)
    bk = np.asarray(inputs["bk"], f32)
    bv = np.asarray(inputs["bv"], f32)
    bo = np.asarray(inputs["bo"], f32)
    bp = np.asarray(inputs["bp"], f32)
    bg = np.asarray(inputs["bg"], f32)
    offset = np.asarray(inputs["offset"], f32)
    gate = np.asarray(inputs["gate"], f32)
    ln1_w = np.asarray(inputs["ln1_w"], f32)
    ln1_b = np.asarray(inputs["ln1_b"], f32)
    ln2_w = np.asarray(inputs["ln2_w"], f32)
    ln2_b = np.asarray(inputs["ln2_b"], f32)

    gv = float(1.0 / (1.0 + np.exp(-gate[0])))

    sel = np.zeros((16, 16, 64), f32)
    for h in range(16):
        sel[h, h, :] = 1.0
    ident = np.eye(128, dtype=f32)

    shared = {
        "wqT": np.ascontiguousarray(Wq.T).astype(_bf),
        "wkT": np.ascontiguousarray(Wk.T).astype(_bf),
        "wvT": np.ascontiguousarray(Wv.T).astype(_bf),
        "woT": np.ascontiguousarray(Wo.T).astype(_bf),
        "wpT": np.ascontiguousarray(Wp.T).astype(_bf),
        "wgT": np.ascontiguousarray(Wg.T).astype(_bf),
        "bqc": np.ascontiguousarray(bq.reshape(8, 128).T),
        "bkc": np.ascontiguousarray(bk.reshape(8, 128).T),
        "bvf": bv,
        "bgf": bg,
        "ln1w": ln1_w,
        "ln1b": ln1_b,
        "ln2w": ln2_w,
        "ln2b": ln2_b,
        "offs": (offset + bp).astype(f32),
        "offc": (np.pi - offset + bp).astype(f32),
        "sel": sel.astype(_bf),
        "ident": ident.astype(_bf),
    }

    in_maps = []
    xT_by_b = [np.ascontiguousarray(x[b].T).astype(_bf) for b in range(B)]
    for c in range(NCORES):
        b, qc = c // 4, c % 4
        xT_b = xT_by_b[b]
        m = dict(shared)
        m["xT"] = xT_b
        m["xqT"] = np.ascontiguousarray(xT_b[:, qc * SC : (qc + 1) * SC])
        m["xres"] = np.ascontiguousarray(x[b, qc * SC : (qc + 1) * SC] + bo)
        in_maps.append(m)
    return in_maps, gv


def run(inputs, trace=False, tmpdir=None):
    """Run the kernel; returns (full_output, BassKernelResults)."""
    from concourse.bass_utils import run_bass_kernel_spmd

    in_maps, gv = _host_inputs(inputs)
    ln_triv = (
        bool(np.all(np.asarray(inputs["ln1_w"]) == 1.0)),
        bool(np.all(np.asarray(inputs["ln1_b"]) == 0.0)),
        bool(np.all(np.asarray(inputs["ln2_w"]) == 1.0)),
        bool(np.all(np.asarray(inputs["ln2_b"]) == 0.0)),
    )
    key = (round(gv, 9), ln_triv)
    if key not in _prog_cache:
        _prog_cache[key] = _build_program(gv, ln_triv)
    nc = _prog_cache[key]
    res = run_bass_kernel_spmd(
        nc, in_maps, core_ids=list(range(NCORES)), trace=trace, tmpdir=tmpdir
    )
    chunks = [res.results[c]["out"] for c in range(NCORES)]
    full = np.concatenate(chunks, axis=0).reshape(B, S, D).astype(np.float32)
    return full, res


def kernel(**inputs) -> np.ndarray:
    out, _ = run(inputs, trace=False)
    return out


# revision 19
# speedup vs baseline: 1.1240x; 1.0110x over previous
"""Trainium2 Bass kernel for nn_FANPhaseOffsetTransformerLayer.

Full inputs -> full output. Sharding: 8 cores; core c handles batch b=c//4
and sequence-row chunk qc=c%4 (512 rows) of that batch. Each core computes
k/v for its whole batch (redundant but fully hidden inside the exp-bound
attention window as PE filler), q only for its row chunk, attention for its
rows over all 16 heads, then Wo/LN1/FAN/LN2 for its rows.

Matmuls run in bf16 (fp32 PSUM accumulation) except the PV product, which
runs in fp8e4m3 with DoubleRow packing (two key blocks per matmul — 2x PE
throughput; probs are positive and bounded ~150, v is ~N(0, 0.4), both well
within e4m3 range). Softmax skips the max-subtraction; the denominator is
a 65th ones-column appended to v. Wo bias is folded into xres on the host;
the FAN gate scale is fused into the residual add; normalize uses bf16.
"""

import math

import numpy as np
import ml_dtypes

B, S, D, H, E = 2, 2048, 1024, 16, 64
P_DIM, G_DIM = 256, 512
SC = 512  # rows per core
NCORES = 8
LN_EPS = 1e-5

_bf = ml_dtypes.bfloat16

_prog_cache = {}


def _build_program(gv: float, ln_triv=(False, False, False, False)):
    from contextlib import ExitStack

    import concourse.bass as bass
    import concourse.bacc as bacc
    import concourse.tile as tile
    import concourse.mybir as mybir

    f32 = mybir.dt.float32
    bf = mybir.dt.bfloat16
    f8 = mybir.dt.float8e4
    AF = mybir.ActivationFunctionType
    ALU = mybir.AluOpType

    nc = bacc.Bacc(
        "TRN2",
        target_bir_lowering=False,
        debug=False,
        enable_asserts=False,
        num_devices=NCORES,
    )

    # ---------------- DRAM I/O ----------------
    d_xT = nc.dram_tensor("xT", [D, S], bf, kind="ExternalInput")
    d_xqT = nc.dram_tensor("xqT", [D, SC], bf, kind="ExternalInput")
    d_xres = nc.dram_tensor("xres", [SC, D], f32, kind="ExternalInput")
    d_wqT = nc.dram_tensor("wqT", [D, D], bf, kind="ExternalInput")
    d_wkT = nc.dram_tensor("wkT", [D, D], bf, kind="ExternalInput")
    d_wvT = nc.dram_tensor("wvT", [D, D], bf, kind="ExternalInput")
    d_woT = nc.dram_tensor("woT", [D, D], bf, kind="ExternalInput")
    d_wpT = nc.dram_tensor("wpT", [D, P_DIM], bf, kind="ExternalInput")
    d_wgT = nc.dram_tensor("wgT", [D, G_DIM], bf, kind="ExternalInput")
    d_bqc = nc.dram_tensor("bqc", [128, 8], f32, kind="ExternalInput")
    d_bkc = nc.dram_tensor("bkc", [128, 8], f32, kind="ExternalInput")
    d_bvf = nc.dram_tensor("bvf", [D], f32, kind="ExternalInput")
    d_bgf = nc.dram_tensor("bgf", [G_DIM], f32, kind="ExternalInput")
    d_ln1w = nc.dram_tensor("ln1w", [D], f32, kind="ExternalInput")
    d_ln1b = nc.dram_tensor("ln1b", [D], f32, kind="ExternalInput")
    d_ln2w = nc.dram_tensor("ln2w", [D], f32, kind="ExternalInput")
    d_ln2b = nc.dram_tensor("ln2b", [D], f32, kind="ExternalInput")
    d_offs = nc.dram_tensor("offs", [P_DIM], f32, kind="ExternalInput")
    d_offc = nc.dram_tensor("offc", [P_DIM], f32, kind="ExternalInput")
    d_sel = nc.dram_tensor("sel", [16, 16, 64], bf, kind="ExternalInput")
    d_ident = nc.dram_tensor("ident", [128, 128], bf, kind="ExternalInput")
    d_out = nc.dram_tensor("out", [SC, D], f32, kind="ExternalOutput")

    def bcast(handle, parts):
        ap_ = handle.ap()
        return bass.AP(
            tensor=ap_.tensor, offset=ap_.offset, ap=[[0, parts]] + list(ap_.ap)
        )

    def chunked(handle, nck, cols):
        ap_ = handle.ap()
        return bass.AP(
            tensor=ap_.tensor,
            offset=ap_.offset,
            ap=[[cols, 128], [128 * cols, nck], [1, cols]],
        )

    with tile.TileContext(nc, pool_alloc_mode="queue") as tc:
        with ExitStack() as ctx:
            misc1 = tc.alloc_tile_pool(name="misc1", bufs=1)

            bqc_sb = misc1.tile([128, 8], f32)
            nc.gpsimd.dma_start(out=bqc_sb, in_=d_bqc.ap())
            bkc_sb = misc1.tile([128, 8], f32)
            nc.gpsimd.dma_start(out=bkc_sb, in_=d_bkc.ap())
            bv_bc = misc1.tile([128, D], f32)
            nc.gpsimd.dma_start(out=bv_bc, in_=bcast(d_bvf, 128))
            eps_sb = misc1.tile([128, 1], f32)
            nc.vector.memset(eps_sb, LN_EPS)

            # persistent attention tiles (right side)
            kv = tc.alloc_tile_pool(name="kv", bufs=1, side="right")
            qT_sb = kv.tile([128, 8, SC], bf)
            kT_sb = kv.tile([128, 8, S], bf)
            # v in fp8, DoubleRow layout: [keys, tb-pair, parity, head, 80pad]
            vaug = kv.tile([128, 8, 2, 16, 80], f8)
            nc.vector.memset(vaug[:, :, :, :, 64:65], 1.0)

            # raw attention output staging (lives through normalize)
            apo = tc.alloc_tile_pool(name="attnp", bufs=1, side="right")
            raw_sb = apo.tile([128, 16, 512], bf)
            den16 = apo.tile([16, 512], bf)
            rec16 = apo.tile([16, 512], bf)

            # weights + x^T (live through attention; right side, freed after)
            qkvw2 = tc.alloc_tile_pool(name="qkvw2", bufs=1, side="right")
            wk_sb = qkvw2.tile([128, 8, D], bf)
            wv_sb = qkvw2.tile([128, 8, D], bf)
            xt_sb = qkvw2.tile([128, 8, S], bf)

            # q-projection operands (left, freed before the post phase)
            qkvw = tc.alloc_tile_pool(name="qkvw", bufs=1)
            wq_sb = qkvw.tile([128, 8, D], bf)
            xq_sb = qkvw.tile([128, 8, SC], bf)
            for kc in range(8):
                nc.scalar.dma_start(
                    out=xq_sb[:, kc, :], in_=d_xqT.ap()[kc * 128 : (kc + 1) * 128, :]
                )
                nc.sync.dma_start(
                    out=wq_sb[:, kc, :], in_=d_wqT.ap()[kc * 128 : (kc + 1) * 128, :]
                )
            nc.sync.dma_start(out=wk_sb, in_=chunked(d_wkT, 8, D))
            nc.gpsimd.dma_start(out=wv_sb, in_=chunked(d_wvT, 8, D))
            for tb in range(4):
                ap_ = d_xT.ap()
                nc.gpsimd.dma_start(
                    out=xt_sb[:, :, tb * 512 : (tb + 1) * 512],
                    in_=bass.AP(
                        tensor=ap_.tensor,
                        offset=tb * 512,
                        ap=[[S, 128], [128 * S, 8], [1, 512]],
                    ),
                )

            # q^T for own chunk: kc-outer over all 8 PSUM banks
            with tc.tile_pool(name="ppq", bufs=1, space="PSUM") as ppq:
                psq = ppq.tile([128, 8, 512], f32, tag="big8", name="psq")
                for kc in range(8):
                    for m in range(8):
                        nc.tensor.matmul(
                            psq[:, m, :],
                            lhsT=wq_sb[:, kc, m * 128 : (m + 1) * 128],
                            rhs=xq_sb[:, kc, :],
                            start=(kc == 0),
                            stop=(kc == 7),
                        )
                for m in range(8):
                    nc.vector.tensor_scalar(
                        out=qT_sb[:, m, :],
                        in0=psq[:, m, :],
                        scalar1=bqc_sb[:, m : m + 1],
                        scalar2=None,
                        op0=ALU.add,
                    )
            qkvw.release()

            # ================= attention (k/v fills interleaved) =================
            with tc.tile_pool(name="fil", bufs=1, space="PSUM") as fil, tc.tile_pool(
                name="ppa", bufs=1, space="PSUM"
            ) as ppa:

                def emit_kt_block(tb, m):
                    ps = fil.tile([128, 512], f32, tag="fp", bufs=2, name="fp")
                    for kc in range(8):
                        nc.tensor.matmul(
                            ps,
                            lhsT=wk_sb[:, kc, m * 128 : (m + 1) * 128],
                            rhs=xt_sb[:, kc, tb * 512 : (tb + 1) * 512],
                            start=(kc == 0),
                            stop=(kc == 7),
                        )
                    nc.vector.tensor_scalar(
                        out=kT_sb[:, m, tb * 512 : (tb + 1) * 512],
                        in0=ps,
                        scalar1=bkc_sb[:, m : m + 1],
                        scalar2=None,
                        op0=ALU.add,
                    )

                def emit_v_block(tb, tm, h2):
                    tcx = tb * 4 + tm  # 128-row key block index (0..15)
                    ps = fil.tile([128, 512], f32, tag="fp", bufs=2, name="fp")
                    for kc in range(8):
                        nc.tensor.matmul(
                            ps,
                            lhsT=xt_sb[
                                :, kc, tb * 512 + tm * 128 : tb * 512 + (tm + 1) * 128
                            ],
                            rhs=wv_sb[:, kc, h2 * 512 : (h2 + 1) * 512],
                            start=(kc == 0),
                            stop=(kc == 7),
                        )
                    nc.vector.tensor_tensor(
                        out=vaug[:, tcx // 2, tcx % 2, h2 * 8 : (h2 + 1) * 8, 0:64],
                        in0=ps,
                        in1=bv_bc[:, h2 * 512 : (h2 + 1) * 512],
                        op=ALU.add,
                    )

                def quarter_fills(q):
                    """16 fill thunks that produce k^T and v for quarter q."""
                    fns = []
                    for m in range(8):
                        fns.append(lambda m=m: emit_kt_block(q, m))
                    for tm in range(4):
                        for h2 in range(2):
                            fns.append(lambda tm=tm, h2=h2: emit_v_block(q, tm, h2))
                    # interleave k and v emissions
                    out = []
                    for a, b2 in zip(fns[:8], fns[8:]):
                        out.append(a)
                        out.append(b2)
                    return out

                def attn_pair(p, q, fills):
                    opsums = []
                    for j in range(2):
                        op = ppa.tile(
                            [65, 512], f32, tag=f"opsum{j}", bufs=1, name=f"opsum{j}"
                        )
                        opsums.append(op)
                    for tp in range(2):
                        tbp = q * 2 + tp
                        probs = apo.tile(
                            [128, 2, 2, 512], f8, tag="probs", bufs=3, name="probs"
                        )
                        for parity in range(2):
                            tb = q * 4 + tp * 2 + parity
                            if fills:
                                fills.pop(0)()
                            ps2 = ppa.tile(
                                [128, 2, 512], f32, tag="ps2", bufs=2, name="ps2"
                            )
                            for j in range(2):
                                off = j * 64
                                nc.tensor.matmul(
                                    ps2[:, j, :],
                                    lhsT=kT_sb[
                                        off : off + 64, p, tb * 128 : (tb + 1) * 128
                                    ],
                                    rhs=qT_sb[off : off + 64, p, :],
                                    start=True,
                                    stop=True,
                                )
                            nc.scalar.activation(
                                out=probs[:, parity, :, :],
                                in_=ps2,
                                func=AF.Exp,
                                scale=1.0 / math.sqrt(E),
                            )
                        for j in range(2):
                            nc.tensor.matmul(
                                opsums[j],
                                lhsT=vaug[:, tbp, :, 2 * p + j, 0:65],
                                rhs=probs[:, :, j, :],
                                start=(tp == 0),
                                stop=(tp == 1),
                                perf_mode=mybir.MatmulPerfMode.DoubleRow,
                            )
                    for j in range(2):
                        if q == 0:
                            nc.vector.tensor_copy(
                                raw_sb[0:65, 2 * p + j, :], opsums[j]
                            )
                        else:
                            nc.vector.tensor_tensor(
                                out=raw_sb[0:65, 2 * p + j, :],
                                in0=opsums[j],
                                in1=raw_sb[0:65, 2 * p + j, :],
                                op=ALU.add,
                            )

                # quarter 0's k/v first, then attention with next-quarter fills
                for f in quarter_fills(0):
                    f()
                for q in range(4):
                    fills = quarter_fills(q + 1) if q < 3 else []
                    for p in range(8):
                        attn_pair(p, q, fills)
                    while fills:
                        fills.pop(0)()
            qkvw2.release()

            # ------- post-phase constants (loaded during attention) -------
            misc2 = tc.alloc_tile_pool(name="misc2", bufs=1)
            ln1w_bc = ln1b_bc = None
            if not (ln_triv[0] and ln_triv[1]):
                ln1w_bc = misc2.tile([128, D], f32)
                nc.gpsimd.dma_start(out=ln1w_bc, in_=bcast(d_ln1w, 128))
                ln1b_bc = misc2.tile([128, D], f32)
                nc.gpsimd.dma_start(out=ln1b_bc, in_=bcast(d_ln1b, 128))
            ln2w_bc = ln2b_bc = None
            if not (ln_triv[2] and ln_triv[3]):
                ln2w_bc = misc2.tile([128, D], f32)
                nc.gpsimd.dma_start(out=ln2w_bc, in_=bcast(d_ln2w, 128))
                ln2b_bc = misc2.tile([128, D], f32)
                nc.gpsimd.dma_start(out=ln2b_bc, in_=bcast(d_ln2b, 128))
            bg_bc = misc2.tile([128, G_DIM], f32)
            nc.gpsimd.dma_start(out=bg_bc, in_=bcast(d_bgf, 128))
            offs_bc = misc2.tile([128, P_DIM], f32)
            nc.gpsimd.dma_start(out=offs_bc, in_=bcast(d_offs, 128))
            offc_bc = misc2.tile([128, P_DIM], f32)
            nc.gpsimd.dma_start(out=offc_bc, in_=bcast(d_offc, 128))
            sel_sb = misc2.tile([16, 16, 64], bf)
            nc.gpsimd.dma_start(out=sel_sb, in_=d_sel.ap())
            ident_sb = misc2.tile([128, 128], bf)
            nc.gpsimd.dma_start(out=ident_sb, in_=d_ident.ap())
            xres_sb = misc2.tile([128, 4, D], f32)
            nc.sync.dma_start(
                out=xres_sb,
                in_=bass.AP(
                    tensor=d_xres.ap().tensor,
                    offset=0,
                    ap=[[D, 128], [128 * D, 4], [1, D]],
                ),
            )
            wo_sb = misc2.tile([128, 8, D], bf)
            nc.sync.dma_start(out=wo_sb, in_=chunked(d_woT, 8, D))
            attn_oT = misc2.tile([128, 8, SC], bf)
            odd_sb = misc2.tile([64, 8, 512], bf)

            # softmax denominators -> reciprocals
            nc.sync.dma_start(out=den16, in_=raw_sb[64:65, :, :])
            with nc.allow_low_precision(reason="softmax denominators: bf16 ample"):
                nc.vector.reciprocal(rec16, den16)

            # normalize; even heads direct, odd heads staged then shifted
            with tc.tile_pool(name="ppn", bufs=2, space="PSUM") as ppn:
                for h in range(16):
                    p_, j = h // 2, h % 2
                    div = ppn.tile([64, 512], f32, tag="div", name="div")
                    nc.tensor.matmul(
                        div,
                        lhsT=sel_sb[:, h, :],
                        rhs=rec16,
                        start=True,
                        stop=True,
                    )
                    if j == 0:
                        out_ap = attn_oT[0:64, p_, :]
                    else:
                        out_ap = odd_sb[0:64, p_, :]
                    nc.vector.tensor_tensor(
                        out=out_ap, in0=raw_sb[0:64, h, :], in1=div, op=ALU.mult
                    )
            nc.sync.dma_start(out=attn_oT[64:128, :, :], in_=odd_sb)
            apo.release()
            kv.release()

            # ================= Wo + LN1 + FAN + LN2 =================
            with tc.tile_pool(name="pw", bufs=1) as pw, tc.tile_pool(
                name="post", bufs=2
            ) as po, tc.tile_pool(name="ppp", bufs=2, space="PSUM") as ppp:
                wp_sb = pw.tile([128, 8, P_DIM], bf)
                nc.sync.dma_start(out=wp_sb, in_=chunked(d_wpT, 8, P_DIM))
                wg_sb = pw.tile([128, 8, G_DIM], bf)
                nc.sync.dma_start(out=wg_sb, in_=chunked(d_wgT, 8, G_DIM))
                z_sb = pw.tile([128, 4, D], f32, tag="zfan", name="z_sb")
                y_sb = pw.tile([128, 4, D], bf)
                yT_sb = pw.tile([128, 8, SC], bf)
                targ_sb = pw.tile([128, 4, 512], f32)
                g_sb = pw.tile([128, 4, 512], f32)

                def ln_stats(z_ap, tag):
                    stats = po.tile([128, 2, 6], f32, tag=f"lnst{tag}", name="lnst")
                    nc.vector.bn_stats(out=stats[:, 0, :], in_=z_ap[:, 0:512])
                    nc.vector.bn_stats(out=stats[:, 1, :], in_=z_ap[:, 512:1024])
                    mv = po.tile([128, 2], f32, tag=f"lnmv{tag}", name="lnmv")
                    nc.vector.bn_aggr(out=mv, in_=stats)
                    sd = po.tile([128, 2], f32, tag=f"lnsd{tag}", name="lnsd")
                    nc.scalar.activation(
                        out=sd[:, 0:1], in_=mv[:, 1:2], func=AF.Sqrt, bias=eps_sb
                    )
                    nc.vector.reciprocal(sd[:, 1:2], sd[:, 0:1])
                    return mv, sd[:, 1:2]

                def ln_apply(z_ap, w_bc, b_bc, out_ap, w_triv, b_triv, tag):
                    mv, rsd = ln_stats(z_ap, tag)
                    if w_triv and b_triv:
                        nc.vector.tensor_scalar(
                            out=out_ap,
                            in0=z_ap,
                            scalar1=mv[:, 0:1],
                            scalar2=rsd,
                            op0=ALU.subtract,
                            op1=ALU.mult,
                        )
                        return
                    tmp = po.tile([128, D], f32, tag=f"lntmp{tag}", name="lntmp")
                    nc.vector.tensor_scalar(
                        out=tmp,
                        in0=z_ap,
                        scalar1=mv[:, 0:1],
                        scalar2=rsd,
                        op0=ALU.subtract,
                        op1=ALU.mult,
                    )
                    if b_triv:
                        nc.vector.tensor_tensor(
                            out=out_ap, in0=tmp, in1=w_bc, op=ALU.mult
                        )
                        return
                    if not w_triv:
                        nc.vector.tensor_tensor(out=tmp, in0=tmp, in1=w_bc, op=ALU.mult)
                    nc.vector.tensor_tensor(out=out_ap, in0=tmp, in1=b_bc, op=ALU.add)

                # Wo projection + residual (+bias already folded into xres)
                for sc in range(4):
                    for h2 in range(2):
                        ps = ppp.tile([128, 512], f32, tag="wops", name="wops")
                        for kc in range(8):
                            nc.tensor.matmul(
                                ps,
                                lhsT=attn_oT[:, kc, sc * 128 : (sc + 1) * 128],
                                rhs=wo_sb[:, kc, h2 * 512 : (h2 + 1) * 512],
                                start=(kc == 0),
                                stop=(kc == 7),
                            )
                        nc.vector.tensor_tensor(
                            out=z_sb[:, sc, h2 * 512 : (h2 + 1) * 512],
                            in0=ps,
                            in1=xres_sb[:, sc, h2 * 512 : (h2 + 1) * 512],
                            op=ALU.add,
                        )
                for sc in range(4):
                    ln_apply(
                        z_sb[:, sc, :], ln1w_bc, ln1b_bc, y_sb[:, sc, :],
                        ln_triv[0], ln_triv[1], "a",
                    )

                # transpose y (bf16)
                for sc in range(4):
                    for dc in range(8):
                        tp = ppp.tile([128, 128], bf, tag="tp", name="tp")
                        nc.tensor.transpose(
                            tp, y_sb[:, sc, dc * 128 : (dc + 1) * 128], ident_sb
                        )
                        nc.vector.tensor_copy(
                            yT_sb[:, dc, sc * 128 : (sc + 1) * 128], tp
                        )

                # FAN sin branch
                RC = 12582912.0  # 1.5 * 2**23
                INV2PI = 1.0 / (2.0 * math.pi)
                for sc in range(4):
                    psp = ppp.tile([128, P_DIM], f32, tag="pps", name="pps")
                    for kc in range(8):
                        nc.tensor.matmul(
                            psp,
                            lhsT=yT_sb[:, kc, sc * 128 : (sc + 1) * 128],
                            rhs=wp_sb[:, kc, :],
                            start=(kc == 0),
                            stop=(kc == 7),
                        )
                    nc.vector.tensor_tensor(
                        out=targ_sb[:, sc, 0:256], in0=psp, in1=offs_bc, op=ALU.add
                    )
                    nc.vector.tensor_tensor(
                        out=targ_sb[:, sc, 256:512], in0=psp, in1=offc_bc, op=ALU.add
                    )
                    nred = po.tile([128, 512], f32, tag="nred", name="nred")
                    nc.vector.tensor_scalar(
                        out=nred,
                        in0=targ_sb[:, sc, :],
                        scalar1=INV2PI,
                        scalar2=RC,
                        op0=ALU.mult,
                        op1=ALU.add,
                    )
                    nc.vector.tensor_scalar(
                        out=nred, in0=nred, scalar1=RC, scalar2=None,
                        op0=ALU.subtract,
                    )
                    nc.vector.scalar_tensor_tensor(
                        out=targ_sb[:, sc, :],
                        in0=nred,
                        scalar=-2.0 * math.pi,
                        in1=targ_sb[:, sc, :],
                        op0=ALU.mult,
                        op1=ALU.add,
                    )
                for sc in range(4):
                    nc.scalar.activation(
                        out=targ_sb[:, sc, :], in_=targ_sb[:, sc, :], func=AF.Sin
                    )

                # FAN gelu branch
                for sc in range(4):
                    psg = ppp.tile([128, G_DIM], f32, tag="ppg", name="ppg")
                    for kc in range(8):
                        nc.tensor.matmul(
                            psg,
                            lhsT=yT_sb[:, kc, sc * 128 : (sc + 1) * 128],
                            rhs=wg_sb[:, kc, :],
                            start=(kc == 0),
                            stop=(kc == 7),
                        )
                    nc.vector.tensor_tensor(
                        out=g_sb[:, sc, :], in0=psg, in1=bg_bc, op=ALU.add
                    )
                for sc in range(4):
                    nc.scalar.activation(
                        out=g_sb[:, sc, :], in_=g_sb[:, sc, :], func=AF.Gelu
                    )

                # z2 = y + gv*sin | y + (1-gv)*gelu, then LN2 + output
                for sc in range(4):
                    z2 = po.tile([128, D], f32, tag="z2", name="z2")
                    nc.vector.scalar_tensor_tensor(
                        out=z2[:, 0:512],
                        in0=targ_sb[:, sc, :],
                        scalar=float(gv),
                        in1=y_sb[:, sc, 0:512],
                        op0=ALU.mult,
                        op1=ALU.add,
                    )
                    nc.vector.scalar_tensor_tensor(
                        out=z2[:, 512:1024],
                        in0=g_sb[:, sc, :],
                        scalar=float(1.0 - gv),
                        in1=y_sb[:, sc, 512:1024],
                        op0=ALU.mult,
                        op1=ALU.add,
                    )
                    outt = po.tile([128, D], f32, tag="outt", name="outt")
                    ln_apply(
                        z2, ln2w_bc, ln2b_bc, outt, ln_triv[2], ln_triv[3], sc % 2
                    )
                    nc.sync.dma_start(
                        out=d_out.ap()[sc * 128 : (sc + 1) * 128, :], in_=outt
                    )

            misc2.release()
            misc1.release()

    nc.compile()
    return nc


def _host_inputs(inputs):
    """Build the per-core in_maps (list of 8 dicts) plus baked gate value."""
    f32 = np.float32
    x = np.asarray(inputs["x"], f32)
    Wq = np.asarray(inputs["Wq"], f32)
    Wk = np.asarray(inputs["Wk"], f32)
    Wv = np.asarray(inputs["Wv"], f32)
    Wo = np.asarray(inputs["Wo"], f32)
    Wp = np.asarray(inputs["Wp"], f32)
    Wg = np.asarray(inputs["Wg"], f32)
    bq = np.asarray(inputs["bq"], f32)
    bk = np.asarray(inputs["bk"], f32)
    bv = np.asarray(inputs["bv"], f32)
    bo = np.asarray(inputs["bo"], f32)
    bp = np.asarray(inputs["bp"], f32)
    bg = np.asarray(inputs["bg"], f32)
    offset = np.asarray(inputs["offset"], f32)
    gate = np.asarray(inputs["gate"], f32)
    ln1_w = np.asarray(inputs["ln1_w"], f32)
    ln1_b = np.asarray(inputs["ln1_b"], f32)
    ln2_w = np.asarray(inputs["ln2_w"], f32)
    ln2_b = np.asarray(inputs["ln2_b"], f32)

    gv = float(1.0 / (1.0 + np.exp(-gate[0])))

    sel = np.zeros((16, 16, 64), f32)
    for h in range(16):
        sel[h, h, :] = 1.0
    ident = np.eye(128, dtype=f32)

    shared = {
        "wqT": np.ascontiguousarray(Wq.T).astype(_bf),
        "wkT": np.ascontiguousarray(Wk.T).astype(_bf),
        "wvT": np.ascontiguousarray(Wv.T).astype(_bf),
        "woT": np.ascontiguousarray(Wo.T).astype(_bf),
        "wpT": np.ascontiguousarray(Wp.T).astype(_bf),
        "wgT": np.ascontiguousarray(Wg.T).astype(_bf),
        "bqc": np.ascontiguousarray(bq.reshape(8, 128).T),
        "bkc": np.ascontiguousarray(bk.reshape(8, 128).T),
        "bvf": bv,
        "bgf": bg,
        "ln1w": ln1_w,
        "ln1b": ln1_b,
        "ln2w": ln2_w,
        "ln2b": ln2_b,
        "offs": (offset + bp).astype(f32),
        "offc": (np.pi - offset + bp).astype(f32),
        "sel": sel.astype(_bf),
        "ident": ident.astype(_bf),
    }

    in_maps = []
    xT_by_b = [np.ascontiguousarray(x[b].T).astype(_bf) for b in range(B)]
    for c in range(NCORES):
        b, qc = c // 4, c % 4
        xT_b = xT_by_b[b]
        m = dict(shared)
        m["xT"] = xT_b
        m["xqT"] = np.ascontiguousarray(xT_b[:, qc * SC : (qc + 1) * SC])
        m["xres"] = np.ascontiguousarray(x[b, qc * SC : (qc + 1) * SC] + bo)
        in_maps.append(m)
    return in_maps, gv


def run(inputs, trace=False, tmpdir=None):
    """Run the kernel; returns (full_output, BassKernelResults)."""
    from concourse.bass_utils import run_bass_kernel_spmd

    in_maps, gv = _host_inputs(inputs)
    ln_triv = (
        bool(np.all(np.asarray(inputs["ln1_w"]) == 1.0)),
        bool(np.all(np.asarray(inputs["ln1_b"]) == 0.0)),
        bool(np.all(np.asarray(inputs["ln2_w"]) == 1.0)),
        bool(np.all(np.asarray(inputs["ln2_b"]) == 0.0)),
    )
    key = (round(gv, 9), ln_triv)
    if key not in _prog_cache:
        _prog_cache[key] = _build_program(gv, ln_triv)
    nc = _prog_cache[key]
    res = run_bass_kernel_spmd(
        nc, in_maps, core_ids=list(range(NCORES)), trace=trace, tmpdir=tmpdir
    )
    chunks = [res.results[c]["out"] for c in range(NCORES)]
    full = np.concatenate(chunks, axis=0).reshape(B, S, D).astype(np.float32)
    return full, res


def kernel(**inputs) -> np.ndarray:
    out, _ = run(inputs, trace=False)
    return out
